# revision 22
# baseline (speedup 1.0000x reference)
"""Trainium2 Bass kernel for a pre-LN transformer block (B=2, S=2048, D=1024,
H=16, HD=64, DFF=4096), SPMD over 8 NeuronCores.

Sharding: no cross-core collectives. Cores 0-3 take batch 0, cores 4-7 batch 1.
Within its batch, core (g = core % 4) owns the interleaved query tokens g::4
(512 of 2048). Every core redundantly computes K/V for its whole batch element
(cheaper than an on-chip all-gather at these sizes), attends its 512 queries
causally, and runs out-proj + FFN for its own tokens. The host reassembles the
full output from the 8 interleaved slices.

Everything on device is feature-major ([d, tokens]); LayerNorm statistics are
computed with ones-vector matmuls on the TensorEngine, softmax denominators
fall out of an AV matmul with a ones-augmented V, and matmuls run in float32r
(full-rate fp32, ~1e-4 relative rounding). K^T is spilled to DRAM (each pair's
K is read exactly once during attention) to fit SBUF.

The causal structure is identical on every core (interleaving makes the k-loop
bounds core-independent); the only per-core data are the input slices and one
[128, 32] 0/1 mask tile for the diagonal.
"""

import sys
import types

import numpy as np

# ---------------------------------------------------------------------------
# NTFF profile hook shim (antenv.axon_hooks is absent on this image; the boot
# code degrades silently without it, which would crash trace=True runs).
if "antenv.axon_hooks" not in sys.modules:
    try:
        import antenv

        _mod = types.ModuleType("antenv.axon_hooks")
        _mod._hook = None

        def _set_hook(h):
            _mod._hook = h

        def _get_hook():
            return _mod._hook

        _mod.set_axon_ntff_profile_hook = _set_hook
        _mod.get_axon_ntff_profile_hook = _get_hook
        sys.modules["antenv.axon_hooks"] = _mod
        antenv.axon_hooks = _mod
        try:
            from trn_agent_boot.trn_boot import _ntff_profile_via_ctypes

            _hook = _ntff_profile_via_ctypes("/opt/axon/libaxon_pjrt.so")
            if _hook is not None:
                _mod._hook = _hook
        except Exception:
            pass
    except Exception:
        pass

import concourse.bass as bass
import concourse.mybir as mybir
import concourse.tile as tile
from concourse import bacc
from concourse.bass_utils import run_bass_kernel_spmd

F32 = mybir.dt.float32
F32R = mybir.dt.float32r
EXP = mybir.ActivationFunctionType.Exp
SQRT = mybir.ActivationFunctionType.Sqrt
COPY = mybir.ActivationFunctionType.Copy
ADD = mybir.AluOpType.add
SUB = mybir.AluOpType.subtract
MULT = mybir.AluOpType.mult
MAX = mybir.AluOpType.max

B, S, D, H, HD, DFF = 2, 2048, 1024, 16, 64, 4096
EPS = 1e-5
NC = 8
NQ = S // 4          # own query tokens per core (512)
DT = D // 128        # 8 d-tiles
FT = DFF // 128      # 32 dff-tiles
KT = S // 128        # 16 k-token tiles
NP = H // 2          # 8 head pairs
CHUNK = 512          # token chunk for LN / K / V
NCH = S // CHUNK     # 4 chunks

_cache = {}


def _build():
    nc = bacc.Bacc("TRN2", target_bir_lowering=False, debug=False, num_devices=NC)

    din = {}
    for name, shape, dt_ in [
        ("xT", [DT, 128, S], F32R),
        ("xqT", [DT, 128, NQ], F32R),
        ("wq", [DT, 128, D], F32R),
        ("wk", [DT, 128, D], F32R),
        ("wv", [DT, 128, D], F32R),
        ("wp", [DT, 128, D], F32R),
        ("w1", [DT, 128, DFF], F32R),
        ("w2", [FT, 128, D], F32R),
        ("bq", [128, NP], F32),
        ("bk", [128, NP], F32),
        ("bv", [1, D], F32),
        ("bp", [128, DT], F32),
        ("b1", [128, FT], F32),
        ("b2", [128, DT], F32),
        ("mask", [128, 32], F32R),
        ("mask2", [128, 7, 256], F32R),
    ]:
        din[name] = nc.dram_tensor(name, shape, dt_, kind="ExternalInput").ap()
    yT = nc.dram_tensor("yT", [DT, 128, NQ], F32, kind="ExternalOutput").ap()
    ktd = nc.dram_tensor("ktd", [NP, 128, S], F32R).ap()  # K^T spill

    with tile.TileContext(nc) as tc:
        with tc.tile_pool(name="persist", bufs=1) as P:
            ones_f = P.tile([128, 1], F32)
            nc.vector.memset(ones_f, 1.0)
            ones = ones_f.bitcast(F32R)
            eps_t = P.tile([1, 1], F32)
            nc.vector.memset(eps_t, EPS)
            bq_t = P.tile([128, NP], F32)
            nc.sync.dma_start(out=bq_t, in_=din["bq"])
            bk_t = P.tile([128, NP], F32)
            nc.sync.dma_start(out=bk_t, in_=din["bk"])
            bv_row = P.tile([1, D], F32)
            nc.sync.dma_start(out=bv_row, in_=din["bv"])
            BV = P.tile([128, D], F32)
            nc.gpsimd.partition_broadcast(BV, bv_row)
            mask_t = P.tile([128, 32], F32R)
            nc.sync.dma_start(out=mask_t, in_=din["mask"])

            def layernorm_chunk(xtiles, n, htiles, psp, sml):
                """Feature-major LN over DT tiles of [128, n]. htiles may be
                the same tiles as xtiles (in-place)."""
                xsq = [
                    sml.tile([128, n], F32R, name=f"xsq{i}", tag="xsq", bufs=2)
                    for i in range(DT)
                ]
                for i in range(DT):
                    nc.vector.tensor_mul(out=xsq[i], in0=xtiles[i], in1=xtiles[i])
                mu_ps = psp.tile([1, n], F32, tag="mu_ps", bufs=2)
                sq_ps = psp.tile([1, n], F32, tag="sq_ps", bufs=2)
                for i in range(DT):
                    nc.tensor.matmul(mu_ps, ones, xtiles[i], start=(i == 0), stop=(i == DT - 1))
                for i in range(DT):
                    nc.tensor.matmul(sq_ps, ones, xsq[i], start=(i == 0), stop=(i == DT - 1))
                mu = sml.tile([1, n], F32, tag="mu", bufs=2)
                nc.scalar.activation(out=mu, in_=mu_ps, func=COPY, scale=1.0 / D)
                musq = sml.tile([1, n], F32, tag="musq", bufs=2)
                nc.vector.tensor_mul(out=musq, in0=mu, in1=mu)
                var = sml.tile([1, n], F32, tag="var", bufs=2)
                nc.vector.scalar_tensor_tensor(
                    out=var, in0=sq_ps, scalar=1.0 / D, in1=musq, op0=MULT, op1=SUB
                )
                std = sml.tile([1, n], F32, tag="std", bufs=2)
                nc.scalar.activation(out=std, in_=var, func=SQRT, bias=eps_t)
                rstd = sml.tile([1, n], F32, tag="rstd", bufs=2)
                nc.vector.reciprocal(out=rstd, in_=std)
                MU = sml.tile([128, n], F32, tag="MU", bufs=1)
                RS = sml.tile([128, n], F32, tag="RS", bufs=1)
                nc.gpsimd.partition_broadcast(MU, mu)
                nc.gpsimd.partition_broadcast(RS, rstd)
                for i in range(DT):
                    nc.vector.tensor_sub(out=htiles[i], in0=xtiles[i], in1=MU)
                    nc.vector.tensor_mul(out=htiles[i], in0=htiles[i], in1=RS)

            from contextlib import ExitStack
            _es_o = ExitStack()
            _es_v = ExitStack()
            with ExitStack() as _es_outer:
                PO = _es_outer.enter_context(tc.tile_pool(name="oT_pool", bufs=1))
                oTt = [PO.tile([128, NQ], F32R, name=f"oTt{p}") for p in range(NP)]
                PV = _es_v.enter_context(tc.tile_pool(name="pool_v", bufs=1))
                Vt = [PV.tile([128, H, 65], F32R, name=f"Vt{t}") for t in range(KT)]
                QTt = [PV.tile([128, NQ], F32R, name=f"QTt{p}") for p in range(NP)]
                for t in range(KT):
                    nc.vector.memset(Vt[t].bitcast(F32)[:, :, 64:65], 1.0)

                # ---- K/V over the full batch sequence, chunk by chunk ----
                with tc.tile_pool(name="kv_sb", bufs=1) as KB, \
                     tc.tile_pool(name="kv_ps", bufs=1, space="PSUM") as KP:
                    wvt = [
                        KB.tile([128, 2, 512], F32R, name=f"wvt{i}", tag=f"wvt{i}", bufs=1)
                        for i in range(DT)
                    ]
                    for i in range(DT):
                        nc.sync.dma_start(out=wvt[i], in_=din["wv"][i].rearrange("p (n c) -> p n c", n=2))
                    for m in range(NCH):
                        xm = [
                            KB.tile([128, CHUNK], F32R, name=f"xm{i}", tag=f"xm{i}", bufs=2)
                            for i in range(DT)
                        ]
                        for i in range(DT):
                            nc.sync.dma_start(
                                out=xm[i], in_=din["xT"][i, :, m * CHUNK:(m + 1) * CHUNK]
                            )
                        layernorm_chunk(xm, CHUNK, xm, KP, KB)  # in-place -> h1

                        for p in range(NP):
                            k_ps = KP.tile([128, CHUNK], F32, tag="k_ps", bufs=2)
                            wkt = KB.tile(
                                [128, DT, 128], F32R, name=f"wkt{p}", tag="wk_s", bufs=1
                            )
                            nc.sync.dma_start(
                                out=wkt,
                                in_=din["wk"][:, :, p * 128:(p + 1) * 128].rearrange(
                                    "i p c -> p i c"
                                ),
                            )
                            for i in range(DT):
                                nc.tensor.matmul(
                                    k_ps, wkt[:, i, :], xm[i], start=(i == 0), stop=(i == DT - 1)
                                )
                            kst = KB.tile([128, CHUNK], F32R, tag="kst", bufs=1)
                            nc.vector.tensor_scalar(
                                out=kst, in0=k_ps, scalar1=bk_t[:, p:p + 1],
                                scalar2=None, op0=ADD,
                            )
                            nc.sync.dma_start(
                                out=ktd[p, :, m * CHUNK:(m + 1) * CHUNK], in_=kst
                            )

                        for tl in range(CHUNK // 128):
                            t = m * (CHUNK // 128) + tl
                            for nh in range(2):
                                v_ps = KP.tile([128, 512], F32, tag="v_ps", bufs=2)
                                for i in range(DT):
                                    nc.tensor.matmul(
                                        v_ps,
                                        xm[i][:, tl * 128:(tl + 1) * 128],
                                        wvt[i][:, nh, :],
                                        start=(i == 0),
                                        stop=(i == DT - 1),
                                    )
                                nc.vector.tensor_add(
                                    out=Vt[t][:, nh * 8:(nh + 1) * 8, 0:64],
                                    in0=v_ps,
                                    in1=BV[:, nh * 512:(nh + 1) * 512].rearrange(
                                        "p (h k) -> p h k", k=64
                                    ),
                                )

                # ---- own-token LN + Q ----
                with tc.tile_pool(name="q_sb", bufs=1) as QB, \
                     tc.tile_pool(name="q_ps", bufs=1, space="PSUM") as QP:
                    xq = [
                        QB.tile([128, NQ], F32R, name=f"xq{i}", tag=f"xq{i}")
                        for i in range(DT)
                    ]
                    for i in range(DT):
                        nc.sync.dma_start(out=xq[i], in_=din["xqT"][i])
                    layernorm_chunk(xq, NQ, xq, QP, QB)  # in-place
                    for p in range(NP):
                        q_ps = QP.tile([128, NQ], F32, tag="q_ps", bufs=2)
                        wqt = QB.tile([128, DT, 128], F32R, name=f"wqt{p}", tag="wq_s", bufs=2)
                        nc.sync.dma_start(
                            out=wqt,
                            in_=din["wq"][:, :, p * 128:(p + 1) * 128].rearrange("i p c -> p i c"),
                        )
                        for i in range(DT):
                            nc.tensor.matmul(
                                q_ps, wqt[:, i, :], xq[i], start=(i == 0), stop=(i == DT - 1)
                            )
                        nc.vector.tensor_scalar(
                            out=QTt[p], in0=q_ps, scalar1=bq_t[:, p:p + 1],
                            scalar2=None, op0=ADD,
                        )

                # ---- attention ----
                if True:
                    with tc.tile_pool(name="at_sb", bufs=1) as AB, \
                         tc.tile_pool(name="at_ps", bufs=1, space="PSUM") as AP_:
                        osb = {}
                        mask2_t = AB.tile([128, 7, 256], F32R, name="mask2_t")
                        nc.sync.dma_start(out=mask2_t, in_=din["mask2"])
                        for p in range(NP):
                            ktp = AB.tile([128, S], F32R, name=f"ktp{p}", tag="ktp", bufs=3)
                            nc.sync.dma_start(out=ktp, in_=ktd[p])
                            o_ps = [
                                AP_.tile([65, NQ], F32, name=f"o_ps{p}_{h}",
                                         tag=f"o_ps{h}", bufs=2)
                                for h in range(2)
                            ]
                            for j in range(KT):
                                nj = max(256, NQ - 32 * j)
                                q0 = NQ - nj
                                sc = [
                                    AP_.tile([128, nj], F32, name=f"sc{p}_{j}_{h}",
                                             tag=f"sc{h}", bufs=2)
                                    for h in range(2)
                                ]
                                att = [
                                    AB.tile([128, nj], F32R, name=f"att{p}_{j}_{h}",
                                            tag=f"att{h}", bufs=2)
                                    for h in range(2)
                                ]
                                for h in range(2):
                                    nc.tensor.matmul(
                                        sc[h],
                                        ktp[64 * h:64 * (h + 1), 128 * j:128 * (j + 1)],
                                        QTt[p][64 * h:64 * (h + 1), q0:NQ],
                                        start=True,
                                        stop=True,
                                        tile_position=(64 * h, 0),
                                    )
                                    nc.scalar.activation(
                                        out=att[h], in_=sc[h], func=EXP, scale=HD ** -0.5
                                    )
                                    if j <= 8:
                                        nc.vector.tensor_mul(
                                            out=att[h][:, 0:32],
                                            in0=att[h][:, 0:32],
                                            in1=mask_t,
                                        )
                                    else:
                                        nc.vector.tensor_mul(
                                            out=att[h][:, 0:32 * (j - 8) + 32],
                                            in0=att[h][:, 0:32 * (j - 8) + 32],
                                            in1=mask2_t[:, j - 9, 0:32 * (j - 8) + 32],
                                        )
                                    nc.tensor.matmul(
                                        o_ps[h][:, q0:NQ],
                                        Vt[j][:, 2 * p + h, :],
                                        att[h],
                                        start=(j == 0),
                                        stop=(j == KT - 1),
                                    )
                            for h in range(2):
                                osb_ph = AB.tile(
                                    [65, NQ], F32, name=f"osb{p}_{h}", tag=f"osb{2*p+h}",
                                    bufs=1,
                                )
                                nc.vector.tensor_copy(out=osb_ph, in_=o_ps[h])
                                osb[2 * p + h] = osb_ph
                        sums_all = AB.tile([2 * NP, NQ], F32, name="sums_all")
                        for ph in range(2 * NP):
                            nc.sync.dma_start(
                                out=sums_all[ph:ph + 1, :], in_=osb[ph][64:65, :]
                            )
                        rall = AB.tile([2 * NP, NQ], F32, name="rall")
                        nc.vector.reciprocal(out=rall, in_=sums_all)
                        for ph in range(2 * NP):
                            p, h = ph // 2, ph % 2
                            rrow = AB.tile([1, NQ], F32, name=f"rrow{ph}", tag="rrow", bufs=4)
                            nc.sync.dma_start(out=rrow, in_=rall[ph:ph + 1, :])
                            R = AB.tile([64, NQ], F32, name=f"R{ph}", tag="Rb", bufs=2)
                            nc.gpsimd.partition_broadcast(R, rrow)
                            nc.vector.tensor_mul(
                                out=oTt[p][64 * h:64 * (h + 1), :],
                                in0=osb[ph][0:64, :],
                                in1=R,
                            )

                    # ---- out-proj + residual -> x1T; LN2 -> h2T; FFN ----
                    _es_v.close()  # free Vt/QTt before out-proj
                    PM = _es_outer.enter_context(tc.tile_pool(name="mid", bufs=1))
                    x1T = [PM.tile([128, NQ], F32R, name=f"x1T{t}") for t in range(DT)]
                    h2T = [PM.tile([128, NQ], F32R, name=f"h2T{t}") for t in range(DT)]
                    if True:
                        with tc.tile_pool(name="op_sb", bufs=1) as OB, \
                             tc.tile_pool(name="op_ps", bufs=1, space="PSUM") as OP:
                            xq2 = [
                                OB.tile([128, NQ], F32R, name=f"xq2{i}", tag=f"xq2{i}")
                                for i in range(DT)
                            ]
                            for i in range(DT):
                                nc.sync.dma_start(out=xq2[i], in_=din["xqT"][i])
                            for t in range(DT):
                                a_ps = OP.tile([128, NQ], F32, tag="a_ps", bufs=2)
                                wpt = OB.tile(
                                    [128, DT, 128], F32R, name=f"wpt{t}", tag="wp_s", bufs=2
                                )
                                nc.sync.dma_start(
                                    out=wpt,
                                    in_=din["wp"][:, :, t * 128:(t + 1) * 128].rearrange(
                                        "i p c -> p i c"
                                    ),
                                )
                                for p in range(NP):
                                    nc.tensor.matmul(
                                        a_ps, wpt[:, p, :], oTt[p], start=(p == 0), stop=(p == NP - 1)
                                    )
                                bp_col = OB.tile([128, 1], F32, name=f"bp{t}", tag="bp_c", bufs=2)
                                nc.sync.dma_start(out=bp_col, in_=din["bp"][:, t:t + 1])
                                nc.vector.scalar_tensor_tensor(
                                    out=x1T[t], in0=a_ps, scalar=bp_col,
                                    in1=xq2[t].bitcast(F32), op0=ADD, op1=ADD,
                                )
                            layernorm_chunk(x1T, NQ, h2T, OP, OB)

                        with tc.tile_pool(name="f_sb", bufs=1) as FB, \
                             tc.tile_pool(name="f_ps", bufs=1, space="PSUM") as FP:
                            fT = [
                                FB.tile([128, NQ], F32R, name=f"fT{f}", tag=f"fT{f}")
                                for f in range(FT)
                            ]
                            b1_t = FB.tile([128, FT], F32)
                            nc.sync.dma_start(out=b1_t, in_=din["b1"])
                            for fg in range(FT // 4):
                                ps4 = [
                                    FP.tile([128, NQ], F32, name=f"f_ps{fg}_{k}",
                                            tag=f"f_ps{k}", bufs=1)
                                    for k in range(4)
                                ]
                                for i in range(DT):
                                    w1t = FB.tile(
                                        [128, 512], F32R, name=f"w1t{fg}_{i}",
                                        tag="w1_s", bufs=3,
                                    )
                                    nc.sync.dma_start(
                                        out=w1t, in_=din["w1"][i, :, fg * 512:(fg + 1) * 512]
                                    )
                                    for k in range(4):
                                        nc.tensor.matmul(
                                            ps4[k],
                                            w1t[:, k * 128:(k + 1) * 128],
                                            h2T[i],
                                            start=(i == 0),
                                            stop=(i == DT - 1),
                                        )
                                for k in range(4):
                                    f = fg * 4 + k
                                    nc.vector.tensor_scalar(
                                        out=fT[f], in0=ps4[k], scalar1=b1_t[:, f:f + 1],
                                        scalar2=0.0, op0=ADD, op1=MAX,
                                    )
                            b2_t = FB.tile([128, DT], F32)
                            nc.sync.dma_start(out=b2_t, in_=din["b2"])
                            for t in range(DT):
                                y_ps = FP.tile([128, NQ], F32, tag="y_ps", bufs=2)
                                for fb in range(FT // 4):
                                    w2t = FB.tile(
                                        [128, 4, 128], F32R, name=f"w2t{t}_{fb}",
                                        tag="w2_s", bufs=3,
                                    )
                                    nc.sync.dma_start(
                                        out=w2t,
                                        in_=din["w2"][4 * fb:4 * fb + 4, :, t * 128:(t + 1) * 128]
                                        .rearrange("f p c -> p f c"),
                                    )
                                    for k in range(4):
                                        f = 4 * fb + k
                                        nc.tensor.matmul(
                                            y_ps, w2t[:, k, :], fT[f], start=(f == 0), stop=(f == FT - 1)
                                        )
                                yt = FB.tile([128, NQ], F32, name=f"yt{t}", tag="yt", bufs=2)
                                nc.vector.scalar_tensor_tensor(
                                    out=yt, in0=y_ps, scalar=b2_t[:, t:t + 1],
                                    in1=x1T[t].bitcast(F32), op0=ADD, op1=ADD,
                                )
                                nc.sync.dma_start(out=yT[t], in_=yt)

    nc.compile()
    return nc


def kernel(**inputs):
    x = np.asarray(inputs["x"], np.float32)
    Wq = np.asarray(inputs["Wq"], np.float32)
    Wk = np.asarray(inputs["Wk"], np.float32)
    Wv = np.asarray(inputs["Wv"], np.float32)
    Wp = np.asarray(inputs["Wp"], np.float32)
    bp = np.asarray(inputs["bp"], np.float32)
    W1 = np.asarray(inputs["W1"], np.float32)
    b1 = np.asarray(inputs["b1"], np.float32)
    W2 = np.asarray(inputs["W2"], np.float32)
    b2 = np.asarray(inputs["b2"], np.float32)
    g1 = np.asarray(inputs["g1"], np.float32)
    beta1 = np.asarray(inputs["beta1"], np.float32)
    g2 = np.asarray(inputs["g2"], np.float32)
    beta2 = np.asarray(inputs["beta2"], np.float32)

    if "nc" not in _cache:
        _cache["nc"] = _build()
    nc = _cache["nc"]

    # ---- host-side weight prep (fold LN affine into the next matmul) ----
    WqF = (Wq * g1[None, :, None]).transpose(1, 0, 2).reshape(D, D)
    WkF = (Wk * g1[None, :, None]).transpose(1, 0, 2).reshape(D, D)
    WvF = (Wv * g1[None, :, None]).transpose(1, 0, 2).reshape(D, D)
    bqv = np.einsum("d,hdk->hk", beta1, Wq).reshape(D)
    bkv = np.einsum("d,hdk->hk", beta1, Wk).reshape(D)
    bvv = np.einsum("d,hdk->hk", beta1, Wv).reshape(D)
    W1F = W1 * g2[:, None]
    b1F = beta2 @ W1 + b1

    def dtiles(w, nt):  # [D_in, N] -> [nt, 128, N]
        return np.ascontiguousarray(w.reshape(nt, 128, -1))

    common = {
        "wq": dtiles(WqF, DT),
        "wk": dtiles(WkF, DT),
        "wv": dtiles(WvF, DT),
        "wp": dtiles(Wp, DT),
        "w1": dtiles(W1F, DT),
        "w2": dtiles(W2, FT),
        "bq": np.ascontiguousarray(bqv.reshape(NP, 128).T),
        "bk": np.ascontiguousarray(bkv.reshape(NP, 128).T),
        "bv": bvv.reshape(1, D),
        "bp": np.ascontiguousarray(bp.reshape(DT, 128).T),
        "b1": np.ascontiguousarray(b1F.reshape(FT, 128).T),
        "b2": np.ascontiguousarray(b2.reshape(DT, 128).T),
    }

    in_maps = []
    for c in range(NC):
        b, g = c // 4, c % 4
        xb = x[b]                      # [S, D]
        xqv = xb[g::4]                 # [NQ, D]
        k_idx = np.arange(128)[:, None]
        u_idx = np.arange(32)[None, :]
        mask = (k_idx <= 4 * u_idx + g).astype(np.float32)
        u2 = np.arange(256)[None, :]
        mask2 = np.stack(
            [
                (k_idx <= 4 * u2 + g + 1024 - 128 * j).astype(np.float32)
                for j in range(9, 16)
            ],
            axis=1,
        )  # [128, 7, 256]
        m = dict(common)
        m["xT"] = np.ascontiguousarray(xb.T.reshape(DT, 128, S))
        m["xqT"] = np.ascontiguousarray(xqv.T.reshape(DT, 128, NQ))
        m["mask"] = mask
        m["mask2"] = np.ascontiguousarray(mask2)
        in_maps.append(m)

    res = run_bass_kernel_spmd(nc, in_maps, list(range(NC)))
    out = np.empty((B, S, D), np.float32)
    for c in range(NC):
        b, g = c // 4, c % 4
        yt = res.results[c]["yT"].reshape(D, NQ)
        out[b, g::4, :] = yt.T
    return out
